# revision 12
# baseline (speedup 1.0000x reference)
"""Capsule-routing kernel for Trainium2 (8 NeuronCores, data-parallel over batch).

Math (u_hat never materialized):
  u_hat[b,j,n,:] = u[b,n,:] @ W_j          (W_j = W[:, j*16:(j+1)*16])
  iter1: c uniform=0.1 -> q1 = G_j @ (0.1*sum_n u)   (host, tiny)
  iter t: logits b[n,j] = u[n,:] @ q[:,j];  q[:,j] = G_j @ R.T[:,j],
          G_j = W_j W_j.T (symmetric, host-precomputed, fp16)
          c = softmax_j(b);  R.T[f,j] = sum_n u.T[f,n] c[n,j]
  out = squash(R3 @ W)   (squash on host -- 64x160 elementwise epilogue)

HW mapping: all u matmuls keep u on the STATIONARY side -- FWL fast-weight
loads stream 128x128 fp16 stationaries at ~27ns/instr while the moving
operand is tiny, so the PE runs at the LDWEIGHTS-issue roofline:
  - logits: stationary u.T chunk [128f,128n]; q_hi and q_lo fp16 matmuls
    (N=10 each) accumulate in PSUM -> b in fp32, no DVE fold needed
  - R:      stationary u chunk [128n,128f], moving c fp16 [128n,10];
    16 chunks accumulate into one PSUM tile = R.T [128f,10] directly
  - q:      10 G_j matmuls [128,128] fp16, moving = R.T cols of a sample
    PAIR (N=2), so one weight load serves two samples
  - final:  o = column-sums of (W * R.T) via ones-matmul, row 0 -> out
Softmax per (n, chunk): DVE max/sub/sum/recip/mul + one ACT exp, e fp16.
Samples are processed in PAIRS through a 6-stage software pipeline
(L2,R2,G,L3,R3,F), one pair-stage per engine-round, emission ordered
oldest-stage-first so no engine FIFO blocks on another's in-flight work.
Precision (validated vs fp64 host sim, rel_err ~8e-3 < 2e-2): u fp16,
q fp16 hi/lo, c/e fp16, G fp16, softmax/accum fp32.
Per-core DMA: 8 samples x (uT + u) fp16 = 8MB over 3 rings, need-ordered.
"""

import os
import sys

import numpy as np

for _p in ("/opt/trn_rl_repo", "/opt/trn_rl_repo/concourse"):
    if _p not in sys.path and os.path.isdir(_p):
        sys.path.insert(0, _p)

import concourse.bass as bass
import concourse.mybir as mybir
import concourse.tile as tile
from concourse import bacc

F32 = mybir.dt.float32
F16 = mybir.dt.float16
AF = mybir.ActivationFunctionType
AX = mybir.AxisListType
ALU = mybir.AluOpType

N_CORES = 8
B_FULL, N, D = 64, 2048, 128
J, DC = 10, 16
JD = J * DC          # 160
NT = N // 128        # 16 chunks of n per sample
B_LOC = B_FULL // N_CORES  # 8 samples per core
NP = B_LOC // 2      # 4 sample pairs per core
EPS = 1e-7
WARMUP_MM = 18


def _bcast(ap, extra):
    """Append step-0 (broadcast) dims to an AP."""
    return bass.AP(tensor=ap.tensor, offset=ap.offset,
                   ap=list(ap.ap) + [[0, n] for n in extra])


def _bcast_at(ap, pos, n):
    """Insert a step-0 (broadcast) dim at position pos of an AP."""
    a = list(ap.ap)
    return bass.AP(tensor=ap.tensor, offset=ap.offset,
                   ap=a[:pos] + [[0, n]] + a[pos:])


def build_program(for_sim=False):
    if for_sim:
        nc = bacc.Bacc(None, target_bir_lowering=False, debug=True)
    else:
        nc = bacc.Bacc(None)

    ut_d = nc.declare_dram_parameter("ut", [B_LOC, D, NT, D], F16,
                                     isOutput=False)
    un_d = nc.declare_dram_parameter("un", [B_LOC, D, NT, D], F16,
                                     isOutput=False)
    g_d = nc.declare_dram_parameter("g", [D, J, D], F16, isOutput=False)
    q1_d = nc.declare_dram_parameter("q1", [D, NP, 2, 2 * J], F16,
                                     isOutput=False)
    w_d = nc.declare_dram_parameter("w", [D, JD], F32, isOutput=False)
    om_d = nc.declare_dram_parameter("ones_mat", [D, D], F16, isOutput=False)
    out_d = nc.declare_dram_parameter("out", [B_LOC, JD], F32, isOutput=True)

    with tile.TileContext(nc) as tc:
        with (
            tc.tile_pool(name="big", bufs=1) as big,
            tc.tile_pool(name="consts", bufs=1) as consts,
            tc.tile_pool(name="sm", bufs=3) as sm,
            tc.tile_pool(name="chain", bufs=3) as chain,
            tc.tile_pool(name="q2p", bufs=3) as q2p,
            tc.tile_pool(name="psumB", bufs=3, space="PSUM") as psumB,
            tc.tile_pool(name="psumR", bufs=2, space="PSUM") as psumR,
            tc.tile_pool(name="psumQ", bufs=1, space="PSUM") as psumQ,
            tc.tile_pool(name="psumO", bufs=2, space="PSUM") as psumO,
        ):
            w_sb = consts.tile([D, JD], F32)
            ones_sb = consts.tile([D, D], F16)
            g_sb = consts.tile([D, J, D], F16)
            q1_sb = consts.tile([D, NP, 2, 2 * J], F16)
            out_sb = consts.tile([1, B_LOC, JD], F32)

            ut = [big.tile([D, NT, D], F16, tag=f"ut{b}", name=f"ut{b}")
                  for b in range(B_LOC)]
            un = [big.tile([D, NT, D], F16, tag=f"un{b}", name=f"un{b}")
                  for b in range(B_LOC)]
            # One HWDGE/SWDGE ring sustains only ~130GB/s; the three rings
            # (sync, scalar, gpsimd) ADD UP to the ~340GB/s HBM cap.  Issue
            # u tiles round-robin across all three in exact need order so
            # arrival order tracks pipeline consumption.  Small consts go
            # first on scalar (cheap issues, needed immediately).
            nc.scalar.dma_start(out=ones_sb[:], in_=om_d[:])
            nc.scalar.dma_start(out=q1_sb[:], in_=q1_d[:])
            nc.scalar.dma_start(out=w_sb[:], in_=w_d[:])
            nc.scalar.dma_start(out=g_sb[:], in_=g_d[:])
            # Each tile is further SPLIT into partition thirds, one slice per
            # ring, so every tile completes at the aggregate rate (~1.6us)
            # in strict global order instead of ~4.5us per-ring serial.
            rings = [nc.sync, nc.gpsimd, nc.scalar]
            cuts = [0, 43, 86, D]
            order = []
            for p in range(NP):
                order += [(ut[2 * p], ut_d[2 * p]),
                          (ut[2 * p + 1], ut_d[2 * p + 1]),
                          (un[2 * p], un_d[2 * p]),
                          (un[2 * p + 1], un_d[2 * p + 1])]
            for t, src in order:
                for r, ring in enumerate(rings):
                    lo, hi = cuts[r], cuts[r + 1]
                    ring.dma_start(out=t[lo:hi], in_=src[lo:hi])

            w_jd = w_sb[:].rearrange("p (j d) -> p j d", j=J)

            # HAM warmup: back-to-back matmuls while the first DMAs land.
            wu_ps = psumO.tile([D, 2 * JD], F32, tag="obc", name="wu_ps")
            for _ in range(WARMUP_MM):
                nc.tensor.matmul(wu_ps[:, 0:D], ones_sb[:], ones_sb[:],
                                 start=True, stop=True)

            q2s = [None] * NP   # fp16 [D, 2, 2J] moving operand per pair
            cs = [None] * NP    # fp16 [D, 2, NT, J] softmax output
            rts = [None] * NP   # R.T PSUM [D, 2, J] of the latest iteration
            rt16s = [None] * NP

            def logits(p, q2ap):
                """PE: 2 samples x 16 chunks x (hi,lo); then softmax ops."""
                bp = psumB.tile([D, 2, NT, J], F32, tag="bp")
                for s in range(2):
                    b = 2 * p + s
                    for t in range(NT):
                        nc.tensor.matmul(bp[:, s, t, :], ut[b][:, t, :],
                                         q2ap[:, s, 0:J],
                                         start=True, stop=False)
                        nc.tensor.matmul(bp[:, s, t, :], ut[b][:, t, :],
                                         q2ap[:, s, J:2 * J],
                                         start=False, stop=True)
                negm = sm.tile([D, 2, NT], F32, tag="negm")
                nc.vector.reduce_max(negm[:], bp[:], axis=AX.X, negate=True)
                bs = sm.tile([D, 2, NT, J], F32, tag="bs")
                nc.vector.tensor_add(bs[:], bp[:], _bcast(negm[:], [J]))
                e = sm.tile([D, 2, NT, J], F16, tag="e")
                nc.scalar.activation(
                    e[:].rearrange("p s t j -> p (s t j)"),
                    bs[:].rearrange("p s t j -> p (s t j)"), AF.Exp)
                z = sm.tile([D, 2, NT], F32, tag="z")
                nc.vector.reduce_sum(z[:], e[:], axis=AX.X)
                zr = sm.tile([D, 2, NT], F32, tag="zr")
                nc.vector.reciprocal(zr[:], z[:])
                c = sm.tile([D, 2, NT, J], F16, tag="c")
                nc.gpsimd.tensor_mul(c[:], e[:], _bcast(zr[:], [J]))
                cs[p] = c

            def r_mm(p):
                """PE: R.T [128f, 2, J] accumulated over 16 chunks/sample."""
                rp = psumR.tile([D, 2, J], F32, tag="rp")
                for s in range(2):
                    b = 2 * p + s
                    for t in range(NT):
                        nc.tensor.matmul(rp[:, s, :], un[b][:, t, :],
                                         cs[p][:, s, t, :],
                                         start=(t == 0), stop=(t == NT - 1))
                rts[p] = rp

            def rt_copy(p):
                rt16 = chain.tile([D, 2, J], F16, tag="rt16")
                nc.scalar.activation(
                    rt16[:].rearrange("p s j -> p (s j)"),
                    rts[p][:].rearrange("p s j -> p (s j)"), AF.Copy)
                rt16s[p] = rt16

            def g_chain(p):
                """q[:,s,j] = G_j @ R.T[:,s,j]; one N=2 matmul per j."""
                qp = psumQ.tile([D, 2, J], F32, tag="qp")
                for j in range(J):
                    nc.tensor.matmul(qp[:, :, j], g_sb[:, j, :],
                                     rt16s[p][:, :, j], start=True, stop=True)
                q2 = q2p.tile([D, 2, 2 * J], F16, tag="q2")
                nc.scalar.activation(q2[:, :, 0:J], qp[:], AF.Copy)
                nc.vector.scalar_tensor_tensor(
                    out=q2[:, :, J:2 * J], in0=qp[:], scalar=1.0,
                    in1=q2[:, :, 0:J], op0=ALU.mult, op1=ALU.subtract)
                q2s[p] = q2

            def final(p):
                """o = colsums(W * R.T) via ones-matmul; row 0 -> out_sb."""
                m1 = chain.tile([D, 2, J, DC], F16, tag="m1")
                nc.vector.tensor_mul(m1[:], _bcast_at(w_jd, 1, 2),
                                     _bcast(rts[p][:], [DC]))
                obc = psumO.tile([D, 2 * JD], F32, tag="obc")
                nc.tensor.matmul(obc[:], ones_sb[:],
                                 m1[:].rearrange("p s j d -> p (s j d)"),
                                 start=True, stop=True)
                nc.scalar.activation(
                    out_sb[0:1, 2 * p:2 * p + 2, :].rearrange(
                        "p s j -> p (s j)"),
                    obc[0:1, :], AF.Copy)

            # 6-stage pipeline over pairs; oldest stage first each round.
            for k in range(NP + 5):
                if 0 <= k - 5 < NP:
                    final(k - 5)
                if 0 <= k - 2 < NP:
                    rt_copy(k - 2)         # ACT early: unblocks G matmuls
                if 0 <= k - 4 < NP:
                    r_mm(k - 4)            # iter-3 R (rts overwritten)
                if 0 <= k - 3 < NP:
                    logits(k - 3, q2s[k - 3][:])   # iter-3 logits
                if 0 <= k - 2 < NP:
                    g_chain(k - 2)
                if 0 <= k - 1 < NP:
                    r_mm(k - 1)            # iter-2 R
                if 0 <= k < NP:
                    logits(k, q1_sb[:, k, :, :])   # iter-2 logits
            nc.sync.dma_start(out=out_d[:].unsqueeze(0), in_=out_sb[:])

    nc.compile()
    return nc


def _hilo16(x):
    hi = x.astype(np.float16)
    lo = (x - hi.astype(np.float32)).astype(np.float16)
    return hi, lo


def _squash(o):
    s2 = (o ** 2).sum(-1, keepdims=True)
    return o * s2 / ((1.0 + s2) * np.sqrt(s2 + EPS))


_NC = None


def _get_nc():
    global _NC
    if _NC is None:
        _NC = build_program()
    return _NC


def run_sharded(u_vecs: np.ndarray, W: np.ndarray, **kw):
    """Shard over 8 cores, run, return (full_output, BassKernelResults)."""
    from concourse.bass_utils import run_bass_kernel_spmd

    u_vecs = np.ascontiguousarray(u_vecs, dtype=np.float32)
    W = np.ascontiguousarray(W, dtype=np.float32)
    assert u_vecs.shape == (B_FULL, N, D) and W.shape == (D, JD)

    nc = _get_nc()
    Wjd = W.reshape(D, J, DC)
    G = np.einsum('fjd,gjd->jfg', Wjd, Wjd).astype(np.float32)  # [J, D, D]
    g16 = np.ascontiguousarray(G.transpose(1, 0, 2)).astype(np.float16)
    ones16 = np.ones((D, D), np.float16)

    in_maps = []
    for k in range(N_CORES):
        us = u_vecs[k * B_LOC:(k + 1) * B_LOC]          # [8, 2048, 128] f32
        u16 = us.astype(np.float16)
        ut = np.ascontiguousarray(
            u16.transpose(0, 2, 1)).reshape(B_LOC, D, NT, D)
        un = np.ascontiguousarray(
            u16.reshape(B_LOC, NT, D, D).transpose(0, 2, 1, 3))
        st01 = 0.1 * us.sum(axis=1)                     # [8, 128] f32
        q1 = np.einsum('jfg,bg->bfj', G, st01)          # [8, 128, 10] f32
        qh, ql = _hilo16(q1)
        q1_hl = np.concatenate([qh, ql], axis=2)        # [8, 128, 20] f16
        q1_arr = np.ascontiguousarray(
            q1_hl.transpose(1, 0, 2)).reshape(D, NP, 2, 2 * J)
        in_maps.append({
            "ut": ut, "un": un, "g": g16, "q1": q1_arr,
            "w": W, "ones_mat": ones16,
        })
    res = run_bass_kernel_spmd(nc, in_maps, core_ids=list(range(N_CORES)), **kw)
    o3 = np.concatenate([res.results[k]["out"] for k in range(N_CORES)], axis=0)
    out = _squash(o3.reshape(B_FULL, J, DC).astype(np.float32))
    return out.astype(np.float32), res


def kernel(u_vecs: np.ndarray, W: np.ndarray) -> np.ndarray:
    out, _ = run_sharded(u_vecs, W)
    return out


# revision 15
# speedup vs baseline: 2.2714x; 2.2714x over previous
"""Capsule-routing kernel for Trainium2 (8 NeuronCores, data-parallel over batch).

Math (u_hat never materialized):
  u_hat[b,j,n,:] = u[b,n,:] @ W_j          (W_j = W[:, j*16:(j+1)*16])
  iter1: c uniform=0.1 -> q1 = G_j @ (0.1*sum_n u)   (host, tiny)
  iter t: logits b[n,j] = u[n,:] @ q[:,j];  q[:,j] = G_j @ R.T[:,j],
          G_j = W_j W_j.T (symmetric, host-precomputed, fp16)
          c = softmax_j(b);  R.T[f,j] = sum_n u.T[f,n] c[n,j]
  out = squash(R3 @ W)   (squash on host -- 64x160 elementwise epilogue)

HW mapping: all u matmuls keep u on the STATIONARY side -- FWL fast-weight
loads stream 128x128 fp16 stationaries at ~27ns/instr while the moving
operand is tiny, so the PE runs at the LDWEIGHTS-issue roofline:
  - logits: stationary u.T chunk [128f,128n]; q_hi and q_lo fp16 matmuls
    (N=10 each) accumulate in PSUM -> b in fp32, no DVE fold needed
  - R:      stationary u chunk [128n,128f], moving c fp16 [128n,10];
    16 chunks accumulate into one PSUM tile = R.T [128f,10] directly
  - q:      10 G_j matmuls [128,128] fp16, moving = R.T cols of a sample
    PAIR (N=2), so one weight load serves two samples
  - final:  o = column-sums of (W * R.T) via ones-matmul, row 0 -> out
Softmax per (n, chunk): DVE max/sub/sum/recip/mul + one ACT exp, e fp16.
Samples are processed in PAIRS through a 6-stage software pipeline
(L2,R2,G,L3,R3,F), one pair-stage per engine-round, emission ordered
oldest-stage-first so no engine FIFO blocks on another's in-flight work.
Precision (validated vs fp64 host sim, rel_err ~8e-3 < 2e-2): u fp16,
q fp16 hi/lo, c/e fp16, G fp16, softmax/accum fp32.
Per-core DMA: 8 samples x (uT + u) fp16 = 8MB over 3 rings, need-ordered.
"""

import os
import sys

import numpy as np

for _p in ("/opt/trn_rl_repo", "/opt/trn_rl_repo/concourse"):
    if _p not in sys.path and os.path.isdir(_p):
        sys.path.insert(0, _p)

import concourse.bass as bass
import concourse.mybir as mybir
import concourse.tile as tile
from concourse import bacc

F32 = mybir.dt.float32
F16 = mybir.dt.float16
AF = mybir.ActivationFunctionType
AX = mybir.AxisListType
ALU = mybir.AluOpType

N_CORES = 8
B_FULL, N, D = 64, 2048, 128
J, DC = 10, 16
JD = J * DC          # 160
NT = N // 128        # 16 chunks of n per sample
B_LOC = B_FULL // N_CORES  # 8 samples per core
NP = B_LOC // 2      # 4 sample pairs per core
EPS = 1e-7
WARMUP_MM = 18


def _bcast(ap, extra):
    """Append step-0 (broadcast) dims to an AP."""
    return bass.AP(tensor=ap.tensor, offset=ap.offset,
                   ap=list(ap.ap) + [[0, n] for n in extra])


def _bcast_at(ap, pos, n):
    """Insert a step-0 (broadcast) dim at position pos of an AP."""
    a = list(ap.ap)
    return bass.AP(tensor=ap.tensor, offset=ap.offset,
                   ap=a[:pos] + [[0, n]] + a[pos:])


def build_program(for_sim=False):
    if for_sim:
        nc = bacc.Bacc(None, target_bir_lowering=False, debug=True)
    else:
        nc = bacc.Bacc(None)

    ut_d = nc.declare_dram_parameter("ut", [B_LOC, D, NT, D], F16,
                                     isOutput=False)
    un_d = nc.declare_dram_parameter("un", [B_LOC, D, NT, D], F16,
                                     isOutput=False)
    g_d = nc.declare_dram_parameter("g", [D, J, D], F16, isOutput=False)
    q1_d = nc.declare_dram_parameter("q1", [D, NP, 2, 2 * J], F16,
                                     isOutput=False)
    w_d = nc.declare_dram_parameter("w", [D, JD], F32, isOutput=False)
    om_d = nc.declare_dram_parameter("ones_mat", [D, D], F16, isOutput=False)
    out_d = nc.declare_dram_parameter("out", [B_LOC, JD], F32, isOutput=True)

    with tile.TileContext(nc) as tc:
        with (
            tc.tile_pool(name="big", bufs=1) as big,
            tc.tile_pool(name="consts", bufs=1) as consts,
            tc.tile_pool(name="sm", bufs=3) as sm,
            tc.tile_pool(name="chain", bufs=3) as chain,
            tc.tile_pool(name="q2p", bufs=3) as q2p,
            tc.tile_pool(name="psumB", bufs=3, space="PSUM") as psumB,
            tc.tile_pool(name="psumR", bufs=2, space="PSUM") as psumR,
            tc.tile_pool(name="psumQ", bufs=1, space="PSUM") as psumQ,
            tc.tile_pool(name="psumO", bufs=2, space="PSUM") as psumO,
        ):
            w_sb = consts.tile([D, JD], F32)
            ones_sb = consts.tile([D, D], F16)
            g_sb = consts.tile([D, J, D], F16)
            q1_sb = consts.tile([D, NP, 2, 2 * J], F16)
            out_sb = consts.tile([1, B_LOC, JD], F32)

            ut = [big.tile([D, NT, D], F16, tag=f"ut{b}", name=f"ut{b}")
                  for b in range(B_LOC)]
            un = [big.tile([D, NT, D], F16, tag=f"un{b}", name=f"un{b}")
                  for b in range(B_LOC)]
            # One HWDGE/SWDGE ring sustains only ~130GB/s; the three rings
            # (sync, scalar, gpsimd) ADD UP to the ~340GB/s HBM cap.  Issue
            # u tiles round-robin across all three in exact need order so
            # arrival order tracks pipeline consumption.  Small consts go
            # first on scalar (cheap issues, needed immediately).
            nc.scalar.dma_start(out=ones_sb[:], in_=om_d[:])
            nc.scalar.dma_start(out=q1_sb[:], in_=q1_d[:])
            nc.scalar.dma_start(out=w_sb[:], in_=w_d[:])
            nc.scalar.dma_start(out=g_sb[:], in_=g_d[:])
            # Each tile is further SPLIT into free-dim (t) thirds, one slice
            # per ring, keeping all 128 partitions per slice (HWDGE engines
            # map to partition groups -- partition slicing starves them).
            # Every tile then completes at the aggregate ~340GB/s rate
            # (~1.6us) in strict global need order.
            rings = [nc.sync, nc.gpsimd, nc.scalar]
            cuts = [0, 6, 11, NT]
            order = []
            for p in range(NP):
                order += [(ut[2 * p], ut_d[2 * p]),
                          (ut[2 * p + 1], ut_d[2 * p + 1]),
                          (un[2 * p], un_d[2 * p]),
                          (un[2 * p + 1], un_d[2 * p + 1])]
            for t, src in order:
                for r, ring in enumerate(rings):
                    lo, hi = cuts[r], cuts[r + 1]
                    ring.dma_start(out=t[:, lo:hi, :], in_=src[:, lo:hi, :])

            w_jd = w_sb[:].rearrange("p (j d) -> p j d", j=J)

            # HAM warmup: back-to-back matmuls while the first DMAs land.
            wu_ps = psumO.tile([D, 2 * JD], F32, tag="obc", name="wu_ps")
            for _ in range(WARMUP_MM):
                nc.tensor.matmul(wu_ps[:, 0:D], ones_sb[:], ones_sb[:],
                                 start=True, stop=True)

            q2s = [None] * NP   # fp16 [D, 2, 2J] moving operand per pair
            cs = [None] * NP    # fp16 [D, 2, NT, J] softmax output
            rts = [None] * NP   # R.T PSUM [D, 2, J] of the latest iteration
            rt16s = [None] * NP

            def logits(p, q2ap):
                """PE: 2 samples x 16 chunks x (hi,lo); then softmax ops."""
                bp = psumB.tile([D, 2, NT, J], F32, tag="bp")
                for s in range(2):
                    b = 2 * p + s
                    for t in range(NT):
                        nc.tensor.matmul(bp[:, s, t, :], ut[b][:, t, :],
                                         q2ap[:, s, 0:J],
                                         start=True, stop=False)
                        nc.tensor.matmul(bp[:, s, t, :], ut[b][:, t, :],
                                         q2ap[:, s, J:2 * J],
                                         start=False, stop=True)
                negm = sm.tile([D, 2, NT], F32, tag="negm")
                nc.vector.reduce_max(negm[:], bp[:], axis=AX.X, negate=True)
                bs = sm.tile([D, 2, NT, J], F32, tag="bs")
                nc.vector.tensor_add(bs[:], bp[:], _bcast(negm[:], [J]))
                e = sm.tile([D, 2, NT, J], F16, tag="e")
                nc.scalar.activation(
                    e[:].rearrange("p s t j -> p (s t j)"),
                    bs[:].rearrange("p s t j -> p (s t j)"), AF.Exp)
                z = sm.tile([D, 2, NT], F32, tag="z")
                nc.vector.reduce_sum(z[:], e[:], axis=AX.X)
                zr = sm.tile([D, 2, NT], F32, tag="zr")
                nc.vector.reciprocal(zr[:], z[:])
                c = sm.tile([D, 2, NT, J], F16, tag="c")
                nc.vector.tensor_mul(c[:], e[:], _bcast(zr[:], [J]))
                cs[p] = c

            def r_mm(p):
                """PE: R.T [128f, 2, J] accumulated over 16 chunks/sample."""
                rp = psumR.tile([D, 2, J], F32, tag="rp")
                for s in range(2):
                    b = 2 * p + s
                    for t in range(NT):
                        nc.tensor.matmul(rp[:, s, :], un[b][:, t, :],
                                         cs[p][:, s, t, :],
                                         start=(t == 0), stop=(t == NT - 1))
                rts[p] = rp

            def rt_copy(p):
                rt16 = chain.tile([D, 2, J], F16, tag="rt16")
                nc.scalar.activation(
                    rt16[:].rearrange("p s j -> p (s j)"),
                    rts[p][:].rearrange("p s j -> p (s j)"), AF.Copy)
                rt16s[p] = rt16

            def g_chain(p):
                """q[:,s,j] = G_j @ R.T[:,s,j]; one N=2 matmul per j."""
                qp = psumQ.tile([D, 2, J], F32, tag="qp")
                for j in range(J):
                    nc.tensor.matmul(qp[:, :, j], g_sb[:, j, :],
                                     rt16s[p][:, :, j], start=True, stop=True)
                q2 = q2p.tile([D, 2, 2 * J], F16, tag="q2")
                nc.scalar.activation(q2[:, :, 0:J], qp[:], AF.Copy)
                nc.vector.scalar_tensor_tensor(
                    out=q2[:, :, J:2 * J], in0=qp[:], scalar=1.0,
                    in1=q2[:, :, 0:J], op0=ALU.mult, op1=ALU.subtract)
                q2s[p] = q2

            def final(p):
                """o = colsums(W * R.T) via ones-matmul; row 0 -> out_sb."""
                m1 = chain.tile([D, 2, J, DC], F16, tag="m1")
                nc.vector.tensor_mul(m1[:], _bcast_at(w_jd, 1, 2),
                                     _bcast(rts[p][:], [DC]))
                obc = psumO.tile([D, 2 * JD], F32, tag="obc")
                nc.tensor.matmul(obc[:], ones_sb[:],
                                 m1[:].rearrange("p s j d -> p (s j d)"),
                                 start=True, stop=True)
                nc.scalar.activation(
                    out_sb[0:1, 2 * p:2 * p + 2, :].rearrange(
                        "p s j -> p (s j)"),
                    obc[0:1, :], AF.Copy)

            # 6-stage pipeline over pairs; oldest stage first each round.
            # L3 right after F so its PSUM (and the iter-3 softmax chain)
            # start as early as possible within the round.
            for k in range(NP + 5):
                if 0 <= k - 5 < NP:
                    final(k - 5)
                if 0 <= k - 2 < NP:
                    rt_copy(k - 2)         # ACT early: unblocks G matmuls
                if 0 <= k - 3 < NP:
                    logits(k - 3, q2s[k - 3][:])   # iter-3 logits
                if 0 <= k - 4 < NP:
                    r_mm(k - 4)            # iter-3 R (rts overwritten)
                if 0 <= k - 2 < NP:
                    g_chain(k - 2)
                if 0 <= k - 1 < NP:
                    r_mm(k - 1)            # iter-2 R
                if 0 <= k < NP:
                    logits(k, q1_sb[:, k, :, :])   # iter-2 logits
            nc.sync.dma_start(out=out_d[:].unsqueeze(0), in_=out_sb[:])

    nc.compile()
    return nc


def _hilo16(x):
    hi = x.astype(np.float16)
    lo = (x - hi.astype(np.float32)).astype(np.float16)
    return hi, lo


def _squash(o):
    s2 = (o ** 2).sum(-1, keepdims=True)
    return o * s2 / ((1.0 + s2) * np.sqrt(s2 + EPS))


_NC = None


def _get_nc():
    global _NC
    if _NC is None:
        _NC = build_program()
    return _NC


def run_sharded(u_vecs: np.ndarray, W: np.ndarray, **kw):
    """Shard over 8 cores, run, return (full_output, BassKernelResults)."""
    from concourse.bass_utils import run_bass_kernel_spmd

    u_vecs = np.ascontiguousarray(u_vecs, dtype=np.float32)
    W = np.ascontiguousarray(W, dtype=np.float32)
    assert u_vecs.shape == (B_FULL, N, D) and W.shape == (D, JD)

    nc = _get_nc()
    Wjd = W.reshape(D, J, DC)
    G = np.einsum('fjd,gjd->jfg', Wjd, Wjd).astype(np.float32)  # [J, D, D]
    g16 = np.ascontiguousarray(G.transpose(1, 0, 2)).astype(np.float16)
    ones16 = np.ones((D, D), np.float16)

    in_maps = []
    for k in range(N_CORES):
        us = u_vecs[k * B_LOC:(k + 1) * B_LOC]          # [8, 2048, 128] f32
        u16 = us.astype(np.float16)
        ut = np.ascontiguousarray(
            u16.transpose(0, 2, 1)).reshape(B_LOC, D, NT, D)
        un = np.ascontiguousarray(
            u16.reshape(B_LOC, NT, D, D).transpose(0, 2, 1, 3))
        st01 = 0.1 * us.sum(axis=1)                     # [8, 128] f32
        q1 = np.einsum('jfg,bg->bfj', G, st01)          # [8, 128, 10] f32
        qh, ql = _hilo16(q1)
        q1_hl = np.concatenate([qh, ql], axis=2)        # [8, 128, 20] f16
        q1_arr = np.ascontiguousarray(
            q1_hl.transpose(1, 0, 2)).reshape(D, NP, 2, 2 * J)
        in_maps.append({
            "ut": ut, "un": un, "g": g16, "q1": q1_arr,
            "w": W, "ones_mat": ones16,
        })
    res = run_bass_kernel_spmd(nc, in_maps, core_ids=list(range(N_CORES)), **kw)
    o3 = np.concatenate([res.results[k]["out"] for k in range(N_CORES)], axis=0)
    out = _squash(o3.reshape(B_FULL, J, DC).astype(np.float32))
    return out.astype(np.float32), res


def kernel(u_vecs: np.ndarray, W: np.ndarray) -> np.ndarray:
    out, _ = run_sharded(u_vecs, W)
    return out
